# revision 1
# baseline (speedup 1.0000x reference)
"""Trainium2 Bass kernel for nn_EnhancedEdgeScorer (gnn_message_passing).

Sharding: data-parallel over nodes (2048/core) and edges (8192/core) on 8
NeuronCores.  Per layer, each core computes K/V for its node shard, the
shards are AllGathered, and each core gathers its nodes' neighbor K/V rows
with dma_gather.  Key algebraic folds:
  - k/v are projected BEFORE the neighbor gather (gather commutes with the
    row-linear projection), turning the reference's (N*M,H)@(H,H) matmuls
    into (N,H)@(H,H).
  - k-bias drops out (softmax shift invariance); v-bias folds into the
    out-projection bias; the 1/sqrt(dh) scale folds into wq/bq.
Everything dense runs on the PE in bf16 with fp32 PSUM accumulation.
"""

import numpy as np
import ml_dtypes
from contextlib import ExitStack

import concourse.bass as bass
from concourse import bacc
import concourse.tile as tile
import concourse.mybir as mybir
from concourse.masks import make_identity
from concourse.bass_utils import run_bass_kernel_spmd

BF16 = mybir.dt.bfloat16
F32 = mybir.dt.float32
I16 = mybir.dt.int16

N, M, H, HEADS, L, E = 16384, 32, 256, 4, 3, 65536
DH = H // HEADS
T, V, CD = 8, 17, 64
TOTAL = H // 2 + 2 * CD + H // 4  # 320
NC = 8
NL = N // NC      # 2048 nodes per core
EL = E // NC      # 8192 edges per core
P = 128
NT = NL // P      # 16 node tiles per core
ET = EL // 512    # 16 edge chunks per core
NEG = -30.0       # additive pad-mask value (exp(-30) ~ 1e-13)

_bf = lambda a: np.ascontiguousarray(a.astype(ml_dtypes.bfloat16))
_f32 = lambda a: np.ascontiguousarray(a.astype(np.float32))


def _wrap16(idx):
    """Flat index list -> [128, len/16] int16 layout dma_gather expects
    (the 16-partition block is replicated for each of the 8 Q7 cores)."""
    idx = np.asarray(idx, dtype=np.int16)
    assert idx.size % 16 == 0
    return np.ascontiguousarray(np.tile(idx.reshape(-1, 16).T, (8, 1)))


# --------------------------------------------------------------------------
# Bass program (SPMD; per-core differences enter only through input data)
# --------------------------------------------------------------------------

def build_program():
    nc = bacc.Bacc(num_devices=NC)

    dp = lambda nm, shp, dt: nc.declare_dram_parameter(nm, list(shp), dt, isOutput=False)

    # ---- weights (same on all cores) ----
    type_tab = dp("type_tab", [T, H // 2], BF16)          # gather-T, elem 128
    cat_tab = dp("cat_tab", [V * V, 2 * CD], BF16)        # combined cat embeds
    dw = dp("dw", [1, H // 4], F32)                       # degree_w row
    db = dp("db", [H // 4], F32)
    projWT = dp("projWT", [3, P, H], BF16)                # proj_w.T in 3 row-chunks (zero padded)
    proj_b = dp("proj_b", [H], F32)
    wqT = dp("wqT", [L, 2, P, H], BF16)                   # (wq*scale).T row-chunks
    bq = dp("bq", [L, H], F32)                            # bq*scale
    wkT = dp("wkT", [L, 2, P, H], BF16)
    wvT = dp("wvT", [L, 2, P, H], BF16)
    woT = dp("woT", [L, 2, P, H], BF16)
    bo = dp("bo", [L, H], F32)                            # out_b + out_w@bv
    w1T = dp("w1T", [4, P, H], BF16)                  # mlp_w1.T eu/ev row-chunks
    w1eT = dp("w1eT", [2, H], BF16)                   # mlp_w1.T edge-feat rows
    b1 = dp("b1", [P, 2], F32)                            # b1 as [128, chunk]
    w2T = dp("w2T", [2, P, H // 2], BF16)
    b2 = dp("b2", [H // 2], F32)
    w3T = dp("w3T", [P, 1], BF16)
    b3 = dp("b3", [1], F32)

    # ---- per-core data ----
    idx_kv = dp("idx_kv", [P, NT * (P * M // 16)], I16)  # m-major ctx idx per node tile
    idx_type = dp("idx_type", [P, NL // 16], I16)
    idx_cat = dp("idx_cat", [P, NL // 16], I16)
    idx_u = dp("idx_u", [P, EL // 16], I16)
    idx_v = dp("idx_v", [P, EL // 16], I16)
    logd = dp("logd", [1, NL], F32)
    kp = dp("kp", [NL, M], F32)                           # additive pad mask (0 / NEG)
    efT = dp("efT", [2, EL], BF16)

    out_d = nc.declare_dram_parameter("out", [EL], F32, isOutput=True)

    # ---- internal DRAM ----
    kloc = nc.dram_tensor("kloc", [NL, H], BF16)
    vloc = nc.dram_tensor("vloc", [NL, H], BF16)
    xloc = nc.dram_tensor("xloc", [NL, H], BF16)
    kall = nc.dram_tensor("kall", [N, H], BF16, addr_space="Shared")
    vall = nc.dram_tensor("vall", [N, H], BF16, addr_space="Shared")
    xall = nc.dram_tensor("xall", [N, H], BF16, addr_space="Shared")

    groups = [list(range(NC))]
    Alu = mybir.AluOpType
    Act = mybir.ActivationFunctionType

    with tile.TileContext(nc) as tc, ExitStack() as ctx:
        const = ctx.enter_context(tc.tile_pool(name="const", bufs=1))
        xpool = ctx.enter_context(tc.tile_pool(name="xpool", bufs=1))

        # ---------------- constants into SBUF ----------------
        gather = nc.gpsimd.dma_gather
        reg_nl = nc.gpsimd.to_reg(NL)
        reg_pm = nc.gpsimd.to_reg(P * M)
        reg_e2 = nc.gpsimd.to_reg(EL // 2)

        ident = const.tile([P, P], BF16)
        make_identity(nc, ident)

        def bcast_row(dram_ap, n, name):
            t = const.tile([P, n], F32, tag=name, name=name)
            src = bass.AP(tensor=dram_ap.tensor, offset=dram_ap.offset,
                          ap=[[0, P]] + dram_ap.ap)
            nc.sync.dma_start(out=t[:], in_=src)
            return t

        pb_b = bcast_row(proj_b[:], H, "pb")
        bq_b = [bcast_row(bq[ll, :], H, f"bq{ll}") for ll in range(L)]
        bo_b = [bcast_row(bo[ll, :], H, f"bo{ll}") for ll in range(L)]

        db_sb = const.tile([H // 4, 1], F32)
        nc.sync.dma_start(out=db_sb[:], in_=db.rearrange("(p o) -> p o", o=1))
        dw_sb = const.tile([1, H // 4], F32)
        nc.sync.dma_start(out=dw_sb[:], in_=dw[:])
        b1_sb = const.tile([P, 2], F32)
        nc.sync.dma_start(out=b1_sb[:], in_=b1[:])
        b2_sb = const.tile([H // 2, 1], F32)
        nc.sync.dma_start(out=b2_sb[:], in_=b2.rearrange("(p o) -> p o", o=1))
        b3_sb = const.tile([1, 1], F32)
        nc.sync.dma_start(out=b3_sb[:], in_=b3.rearrange("(p o) -> p o", o=1))

        ikv_sb = const.tile([P, NT * P * M // 16], I16)
        nc.sync.dma_start(out=ikv_sb[:], in_=idx_kv[:])
        ity_sb = const.tile([P, NL // 16], I16)
        nc.sync.dma_start(out=ity_sb[:], in_=idx_type[:])
        ica_sb = const.tile([P, NL // 16], I16)
        nc.sync.dma_start(out=ica_sb[:], in_=idx_cat[:])
        iu_sb = const.tile([P, EL // 16], I16)
        nc.sync.dma_start(out=iu_sb[:], in_=idx_u[:])
        iv_sb = const.tile([P, EL // 16], I16)
        nc.sync.dma_start(out=iv_sb[:], in_=idx_v[:])

        kp_sb = const.tile([P, NT, M], F32)
        nc.sync.dma_start(out=kp_sb[:], in_=kp.rearrange("(t p) m -> p t m", p=P))
        logd_sb = const.tile([1, NL], F32)
        nc.sync.dma_start(out=logd_sb[:], in_=logd[:])

        pw_sb = const.tile([P, 3, H], BF16)
        nc.sync.dma_start(out=pw_sb[:], in_=projWT.rearrange("c p o -> p c o"))
        w1_sb = const.tile([P, 4, H], BF16)
        nc.sync.dma_start(out=w1_sb[:], in_=w1T.rearrange("c p o -> p c o"))
        w1e_sb = const.tile([2, H], BF16)
        nc.sync.dma_start(out=w1e_sb[:], in_=w1eT[:])
        w2_sb = const.tile([P, 2, H // 2], BF16)
        nc.sync.dma_start(out=w2_sb[:], in_=w2T.rearrange("c p o -> p c o"))
        w3_sb = const.tile([P, 1], BF16)
        nc.sync.dma_start(out=w3_sb[:], in_=w3T[:])

        x_sb = xpool.tile([P, NT, H], BF16)

        # ---------------- node feature encoding (scoped pools) ----------------
        with ExitStack() as ectx:
            enc = ectx.enter_context(tc.tile_pool(name="enc", bufs=1))
            epsum = ectx.enter_context(tc.tile_pool(name="epsum", bufs=2, space="PSUM"))
            teT = enc.tile([P, NL], BF16)
            gather(teT.rearrange("p (c n) -> p c n", c=1), type_tab[:],
                                 ity_sb[:], NL, reg_nl, H // 2, transpose=True, single_packet=False)
            ccT = enc.tile([P, NL], BF16)
            gather(ccT.rearrange("p (c n) -> p c n", c=1), cat_tab[:],
                                 ica_sb[:], NL, reg_nl, 2 * CD, transpose=True, single_packet=False)
            deT = enc.tile([P, NL], BF16)
            nc.vector.memset(deT[:], 0.0)
            for s in range(NL // 512):
                pd = epsum.tile([H // 4, 512], F32, tag="pdeg", name="pd")
                nc.tensor.matmul(pd[:], dw_sb[:], logd_sb[:, s * 512:(s + 1) * 512],
                                 start=True, stop=True)
                nc.scalar.activation(deT[0:H // 4, s * 512:(s + 1) * 512], pd[:],
                                     Act.Relu, bias=db_sb[:])
            for g in range(NT):
                px = epsum.tile([P, H], F32, tag="px", name="px")
                cs = slice(g * P, (g + 1) * P)
                nc.tensor.matmul(px[:], teT[:, cs], pw_sb[:, 0, :], start=True, stop=False)
                nc.tensor.matmul(px[:], ccT[:, cs], pw_sb[:, 1, :], start=False, stop=False)
                nc.tensor.matmul(px[:], deT[:, cs], pw_sb[:, 2, :], start=False, stop=True)
                nc.vector.tensor_tensor(x_sb[:, g, :], px[:], pb_b[:], op=Alu.add)

        work = ctx.enter_context(tc.tile_pool(name="work", bufs=1))
        gath = ctx.enter_context(tc.tile_pool(name="gath", bufs=2))
        att = ctx.enter_context(tc.tile_pool(name="att", bufs=2))
        psum = ctx.enter_context(tc.tile_pool(name="psum", bufs=2, space="PSUM"))
        psum1 = ctx.enter_context(tc.tile_pool(name="psum1", bufs=2, space="PSUM"))

        # ---------------- attention layers ----------------
        for ll in range(L):
            wq_sb = work.tile([P, 2, H], BF16, tag="wq", name="wq")
            wk_sb = work.tile([P, 2, H], BF16, tag="wk", name="wk")
            wv_sb = work.tile([P, 2, H], BF16, tag="wv", name="wv")
            wo_sb = work.tile([P, 2, H], BF16, tag="wo", name="wo")
            nc.sync.dma_start(out=wq_sb[:], in_=wqT[ll].rearrange("c p o -> p c o"))
            nc.sync.dma_start(out=wk_sb[:], in_=wkT[ll].rearrange("c p o -> p c o"))
            nc.sync.dma_start(out=wv_sb[:], in_=wvT[ll].rearrange("c p o -> p c o"))
            nc.sync.dma_start(out=wo_sb[:], in_=woT[ll].rearrange("c p o -> p c o"))

            # x^T tiles (lhsT for projections)
            xT = work.tile([P, 2, NT, P], BF16, tag="xT", name="xT")
            for g in range(NT):
                for c in range(2):
                    pt = psum1.tile([P, P], BF16, tag="ptr", name="pt")
                    nc.tensor.transpose(pt[:], x_sb[:, g, c * P:(c + 1) * P], ident[:])
                    nc.scalar.activation(xT[:, c, g, :], pt[:], Act.Copy)

            q_sb = work.tile([P, NT, H], BF16, tag="q", name="q_sb")
            kall_pview = kloc.rearrange("(t p) o -> p t o", p=P)
            vall_pview = vloc.rearrange("(t p) o -> p t o", p=P)
            for g in range(NT):
                pq = psum.tile([P, H], F32, tag="pmm", name="pq")
                nc.tensor.matmul(pq[:], xT[:, 0, g, :], wq_sb[:, 0, :], start=True, stop=False)
                nc.tensor.matmul(pq[:], xT[:, 1, g, :], wq_sb[:, 1, :], start=False, stop=True)
                nc.vector.tensor_tensor(q_sb[:, g, :], pq[:], bq_b[ll][:], op=Alu.add)
                pk = psum.tile([P, H], F32, tag="pmm", name="pk")
                nc.tensor.matmul(pk[:], xT[:, 0, g, :], wk_sb[:, 0, :], start=True, stop=False)
                nc.tensor.matmul(pk[:], xT[:, 1, g, :], wk_sb[:, 1, :], start=False, stop=True)
                kev = work.tile([P, H], BF16, tag="kev", name="kev", bufs=2)
                nc.scalar.activation(kev[:], pk[:], Act.Copy)
                nc.sync.dma_start(out=kall_pview[:, g, :], in_=kev[:])
                pv = psum.tile([P, H], F32, tag="pmm", name="pv")
                nc.tensor.matmul(pv[:], xT[:, 0, g, :], wv_sb[:, 0, :], start=True, stop=False)
                nc.tensor.matmul(pv[:], xT[:, 1, g, :], wv_sb[:, 1, :], start=False, stop=True)
                vev = work.tile([P, H], BF16, tag="vev", name="vev", bufs=2)
                nc.scalar.activation(vev[:], pv[:], Act.Copy)
                nc.sync.dma_start(out=vall_pview[:, g, :], in_=vev[:])

            nc.gpsimd.collective_compute("AllGather", Alu.bypass, replica_groups=groups,
                                         ins=[kloc[:]], outs=[kall[:]])
            nc.gpsimd.collective_compute("AllGather", Alu.bypass, replica_groups=groups,
                                         ins=[vloc[:]], outs=[vall[:]])

            for t in range(NT):
                isl = ikv_sb[:, t * (P * M // 16):(t + 1) * (P * M // 16)]
                kg = gath.tile([P, M, H], BF16, tag="kg", name="kg")
                gather(kg[:], kall[:], isl, P * M, reg_pm, H, single_packet=False)
                vg = gath.tile([P, M, H], BF16, tag="vg", name="vg")
                gather(vg[:], vall[:], isl, P * M, reg_pm, H, single_packet=False)

                # scores: s[n,m,h] = sum_d q*k  (d-tree reduce, ping-pong pp<->ta)
                pp = att.tile([P, M, HEADS, DH], BF16, tag="pp", name="pp")
                qb = q_sb[:, t, None, :].to_broadcast([P, M, H])
                nc.vector.tensor_tensor(pp.rearrange("p m h d -> p m (h d)"),
                                        kg.rearrange("p m o -> p m o"), qb, op=Alu.mult)
                ta = att.tile([P, M, HEADS, DH // 2], BF16, tag="ta", name="ta")
                nc.vector.tensor_tensor(ta[:], pp[:, :, :, 0:32], pp[:, :, :, 32:64], op=Alu.add)
                nc.vector.tensor_tensor(pp[:, :, :, 0:16], ta[:, :, :, 0:16], ta[:, :, :, 16:32], op=Alu.add)
                nc.vector.tensor_tensor(ta[:, :, :, 0:8], pp[:, :, :, 0:8], pp[:, :, :, 8:16], op=Alu.add)
                nc.vector.tensor_tensor(pp[:, :, :, 0:4], ta[:, :, :, 0:4], ta[:, :, :, 4:8], op=Alu.add)
                nc.vector.tensor_tensor(ta[:, :, :, 0:2], pp[:, :, :, 0:2], pp[:, :, :, 2:4], op=Alu.add)
                s_m = att.tile([P, M, HEADS], F32, tag="sm", name="s_m")
                nc.vector.tensor_tensor(s_m[:], ta[:, :, :, 0], ta[:, :, :, 1], op=Alu.add)

                kpb = kp_sb[:, t, :, None].to_broadcast([P, M, HEADS])
                nc.vector.tensor_tensor(s_m[:], s_m[:], kpb, op=Alu.add)
                es = att.tile([P, M, HEADS], F32, tag="es", name="es")
                nc.scalar.activation(es[:], s_m[:], Act.Exp)
                sums = att.tile([P, HEADS], F32, tag="sums", name="sums")
                nc.vector.tensor_reduce(sums[:], es.rearrange("p m h -> p h m"),
                                        axis=mybir.AxisListType.X, op=Alu.add)
                rs = att.tile([P, HEADS], F32, tag="rs", name="rs")
                nc.vector.reciprocal(rs[:], sums[:])
                attw = att.tile([P, M, HEADS], BF16, tag="attw", name="attw")
                nc.vector.tensor_tensor(attw[:], es[:],
                                        rs[:, None, :].to_broadcast([P, M, HEADS]), op=Alu.mult)

                # AV: o[n,:] = sum_m attw * v  (m-tree, ping-pong av<->tm)
                av = att.tile([P, M, H], BF16, tag="pp", name="av")
                nc.vector.tensor_tensor(av.rearrange("p m (h d) -> p m h d", h=HEADS),
                                        vg.rearrange("p m (h d) -> p m h d", h=HEADS),
                                        attw[:, :, :, None].to_broadcast([P, M, HEADS, DH]),
                                        op=Alu.mult)
                tm = att.tile([P, M // 2, H], BF16, tag="ta", name="tm")
                nc.vector.tensor_tensor(tm[:], av[:, 0:16, :], av[:, 16:32, :], op=Alu.add)
                nc.vector.tensor_tensor(av[:, 0:8, :], tm[:, 0:8, :], tm[:, 8:16, :], op=Alu.add)
                nc.vector.tensor_tensor(tm[:, 0:4, :], av[:, 0:4, :], av[:, 4:8, :], op=Alu.add)
                nc.vector.tensor_tensor(av[:, 0:2, :], tm[:, 0:2, :], tm[:, 2:4, :], op=Alu.add)
                o_sb = att.tile([P, H], BF16, tag="o", name="o_sb")
                nc.vector.tensor_tensor(o_sb[:], av[:, 0, :], av[:, 1, :], op=Alu.add)

                # out-proj + relu -> x
                oT = att.tile([P, 2, P], BF16, tag="oT", name="oT")
                for c in range(2):
                    pt = psum1.tile([P, P], BF16, tag="ptr", name="pt")
                    nc.tensor.transpose(pt[:], o_sb[:, c * P:(c + 1) * P], ident[:])
                    nc.scalar.activation(oT[:, c, :], pt[:], Act.Copy)
                pxn = psum.tile([P, H], F32, tag="pmm", name="pxn")
                nc.tensor.matmul(pxn[:], oT[:, 0, :], wo_sb[:, 0, :], start=True, stop=False)
                nc.tensor.matmul(pxn[:], oT[:, 1, :], wo_sb[:, 1, :], start=False, stop=True)
                nc.vector.tensor_tensor(x_sb[:, t, :], pxn[:], bo_b[ll][:], op=Alu.add)
                nc.vector.tensor_scalar_max(x_sb[:, t, :], x_sb[:, t, :], 0.0)

        # ---------------- edge MLP ----------------
        nc.sync.dma_start(out=xloc.rearrange("(t p) o -> p t o", p=P), in_=x_sb[:])
        nc.gpsimd.collective_compute("AllGather", Alu.bypass, replica_groups=groups,
                                     ins=[xloc[:]], outs=[xall[:]])

        EH = EL // 2
        for half in range(2):
            hsl = slice(half * (EH // 16), (half + 1) * (EH // 16))
            ug = gath.tile([P, 2, EH], BF16, tag="kg", name="ug")
            gather(ug[:], xall[:], iu_sb[:, hsl], EH, reg_e2, H,
                                 transpose=True, single_packet=False)
            vg2 = gath.tile([P, 2, EH], BF16, tag="vg", name="vg2")
            gather(vg2[:], xall[:], iv_sb[:, hsl], EH, reg_e2, H,
                                 transpose=True, single_packet=False)
            for e in range(EH // 512):
                eg = half * (EH // 512) + e
                esl = slice(e * 512, (e + 1) * 512)
                ef_sb = att.tile([2, 512], BF16, tag="ef", name="ef_sb")
                nc.sync.dma_start(out=ef_sb[:], in_=efT[:, eg * 512:(eg + 1) * 512])
                h1T = att.tile([P, 2, 512], BF16, tag="h1T", name="h1T")
                for oc in range(2):
                    ph = psum.tile([P, 512], F32, tag="pbig", name="ph")
                    ocs = slice(oc * P, (oc + 1) * P)
                    nc.tensor.matmul(ph[:], w1_sb[:, 0, ocs], ug[:, 0, esl], start=True, stop=False)
                    nc.tensor.matmul(ph[:], w1_sb[:, 1, ocs], ug[:, 1, esl], start=False, stop=False)
                    nc.tensor.matmul(ph[:], w1_sb[:, 2, ocs], vg2[:, 0, esl], start=False, stop=False)
                    nc.tensor.matmul(ph[:], w1_sb[:, 3, ocs], vg2[:, 1, esl], start=False, stop=False)
                    nc.tensor.matmul(ph[:], w1e_sb[:, ocs], ef_sb[:], start=False, stop=True)
                    nc.scalar.activation(h1T[:, oc, :], ph[:], Act.Relu, bias=b1_sb[:, oc:oc + 1])
                ph2 = psum.tile([P, 512], F32, tag="pbig", name="ph2")
                nc.tensor.matmul(ph2[0:H // 2, :], w2_sb[:, 0, :], h1T[:, 0, :], start=True, stop=False)
                nc.tensor.matmul(ph2[0:H // 2, :], w2_sb[:, 1, :], h1T[:, 1, :], start=False, stop=True)
                h2T = att.tile([H // 2, 512], BF16, tag="h2T", name="h2T")
                nc.scalar.activation(h2T[:], ph2[0:H // 2, :], Act.Relu, bias=b2_sb[:])
                pl = psum1.tile([1, 512], F32, tag="pl", name="pl")
                nc.tensor.matmul(pl[:], w3_sb[:, :], h2T[:], start=True, stop=True)
                lo = att.tile([1, 512], F32, tag="lo", name="lo")
                nc.scalar.activation(lo[:], pl[:], Act.Identity, bias=b3_sb[:])
                nc.sync.dma_start(out=out_d.rearrange("(a b) -> a b", a=ET)[eg, None, :], in_=lo[:])

    nc.finalize()
    return nc


# --------------------------------------------------------------------------
# Host-side prep + runner
# --------------------------------------------------------------------------

_CACHE = {}


def _prep_maps(inputs):
    f = {k: np.asarray(v) for k, v in inputs.items()}
    scale = 1.0 / np.sqrt(np.float32(DH))

    cat0, cat1 = f["cat_embed0"].astype(np.float32), f["cat_embed1"].astype(np.float32)
    cat_tab = np.zeros((V * V, 2 * CD), np.float32)
    for i0 in range(V):
        for i1 in range(V):
            cat_tab[i0 * V + i1] = np.concatenate([cat0[i0], cat1[i1]])

    projWT = np.zeros((3, P, H), np.float32)
    pwt = f["proj_w"].astype(np.float32).T  # [320, 256]
    projWT[0] = pwt[0:128]
    projWT[1] = pwt[128:256]
    projWT[2, 0:64] = pwt[256:320]

    wqT = np.empty((L, 2, P, H), np.float32)
    wkT = np.empty((L, 2, P, H), np.float32)
    wvT = np.empty((L, 2, P, H), np.float32)
    woT = np.empty((L, 2, P, H), np.float32)
    bq = np.empty((L, H), np.float32)
    bo = np.empty((L, H), np.float32)
    for ll in range(L):
        w = f["in_proj_w"][ll].astype(np.float32)
        b = f["in_proj_b"][ll].astype(np.float32)
        wq, wk, wv = w[0:H], w[H:2 * H], w[2 * H:3 * H]
        bq[ll] = b[0:H] * scale
        bv = b[2 * H:3 * H]
        for c in range(2):
            wqT[ll, c] = (wq * scale).T[c * P:(c + 1) * P]
            wkT[ll, c] = wk.T[c * P:(c + 1) * P]
            wvT[ll, c] = wv.T[c * P:(c + 1) * P]
            woT[ll, c] = f["out_w"][ll].astype(np.float32).T[c * P:(c + 1) * P]
        bo[ll] = f["out_b"][ll].astype(np.float32) + f["out_w"][ll].astype(np.float32) @ bv

    w1 = f["mlp_w1"].astype(np.float32)      # [256, 514]
    w1T_full = w1.T                           # [514, 256]
    w1T = np.stack([w1T_full[c * P:(c + 1) * P] for c in range(4)])
    w1eT = w1T_full[512:514]
    b1 = f["mlp_b1"].astype(np.float32).reshape(2, P).T  # [128, 2]
    w2T = np.stack([f["mlp_w2"].astype(np.float32).T[c * P:(c + 1) * P] for c in range(2)])
    w3T = f["mlp_w3"].astype(np.float32).T   # [128, 1]

    shared = {
        "type_tab": _bf(f["type_embed"]),
        "cat_tab": _bf(cat_tab),
        "dw": _f32(f["degree_w"].reshape(1, -1)),
        "db": _f32(f["degree_b"]),
        "projWT": _bf(projWT),
        "proj_b": _f32(f["proj_b"]),
        "wqT": _bf(wqT), "bq": _f32(bq),
        "wkT": _bf(wkT), "wvT": _bf(wvT),
        "woT": _bf(woT), "bo": _f32(bo),
        "w1T": _bf(w1T), "w1eT": _bf(w1eT), "b1": _f32(b1),
        "w2T": _bf(w2T), "b2": _f32(f["mlp_b2"]),
        "w3T": _bf(w3T), "b3": _f32(f["mlp_b3"]),
    }

    ctx = f["context_indices"].astype(np.int64)
    kpm = f["key_padding_mask"].astype(bool)
    maps = []
    for c in range(NC):
        ns = slice(c * NL, (c + 1) * NL)
        es = slice(c * EL, (c + 1) * EL)
        ctx_c = ctx[ns]  # [2048, 32]
        idx_kv = np.concatenate(
            [ctx_c[t * P:(t + 1) * P].T.flatten() for t in range(NT)])
        m = dict(shared)
        m["idx_kv"] = _wrap16(idx_kv)
        m["idx_type"] = _wrap16(f["type_idx"][ns])
        m["idx_cat"] = _wrap16(f["cat_idx"][ns, 0] * V + f["cat_idx"][ns, 1])
        m["idx_u"] = _wrap16(f["u_idx"][es])
        m["idx_v"] = _wrap16(f["v_idx"][es])
        m["logd"] = _f32(f["log_degree"][ns].reshape(1, NL))
        m["kp"] = _f32(np.where(kpm[ns], NEG, 0.0))
        m["efT"] = _bf(f["edge_feats"][es].T)
        maps.append(m)
    return maps


def kernel(**inputs):
    if "nc" not in _CACHE:
        _CACHE["nc"] = build_program()
    nc = _CACHE["nc"]
    maps = _prep_maps(inputs)
    res = run_bass_kernel_spmd(nc, maps, core_ids=list(range(NC)))
    return np.concatenate([res.results[c]["out"] for c in range(NC)]).astype(np.float32)


if __name__ == "__main__":
    nc = build_program()
    print("program built OK")



# revision 8
# speedup vs baseline: 1.5656x; 1.5656x over previous
"""Trainium2 Bass kernel for nn_EnhancedEdgeScorer (gnn_message_passing).

Sharding: data-parallel over nodes (2048/core) and edges (8192/core) on 8
NeuronCores.  Per layer, each core projects K|V (fused, one 512-wide row
per node) for its node shard, AllGathers the shard, and gathers its nodes'
neighbor K|V rows with ONE dma_gather per node tile (desc-gen on the Q7
cores is the serial bottleneck: ~8ns/index, so fusing K and V halves it).
Nodes are sorted by context length on the host and tiles use a per-tile
neighbor count M_t in {4,8,16,32}, cutting gather indices and attention
math by ~27%.

Algebraic folds:
  - k/v projected BEFORE the neighbor gather (gather commutes with the
    row-linear projection).
  - k-bias drops (softmax shift invariance); v-bias folds into the
    out-proj bias; 1/sqrt(dh) folds into wq/bq.
  - softmax normalization deferred to after the AV-sum (o *= 1/denom).
  - x kept TRANSPOSED (feature-major) between layers: projections consume
    xT directly as lhsT, the out-projection computes xT for the next layer
    directly (lhsT=wo^T blocks, rhs=o^T), and its bias+relu runs on the
    Act engine with a per-partition bias column.
  - exp runs on the Act engine with a broadcast input that also EXPANDS
    scores across the head dim, so the AV multiply reads a packed bf16
    operand (2x DVE mode) instead of a stride-0 broadcast (1x mode).
"""

import numpy as np
import ml_dtypes
from contextlib import ExitStack

import concourse.bass as bass
from concourse import bacc
import concourse.tile as tile
import concourse.mybir as mybir
from concourse.masks import make_identity
from concourse.bass_utils import run_bass_kernel_spmd

BF16 = mybir.dt.bfloat16
F32 = mybir.dt.float32
I16 = mybir.dt.int16

N, M, H, HEADS, L, E = 16384, 32, 256, 4, 3, 65536
DH = H // HEADS
T, V, CD = 8, 17, 64
TOTAL = H // 2 + 2 * CD + H // 4  # 320
NC = 8
NL = N // NC      # 2048 nodes per core
EL = E // NC      # 8192 edges per core
P = 128
NT = NL // P      # 16 node tiles per core
ET = EL // 512    # 16 edge chunks per core
NEG = -30.0       # additive pad-mask value (exp(-30) ~ 1e-13)

_bf = lambda a: np.ascontiguousarray(a.astype(ml_dtypes.bfloat16))
_f32 = lambda a: np.ascontiguousarray(a.astype(np.float32))


def _wrap16(idx):
    """Flat index list -> [128, len/16] int16 layout dma_gather expects
    (the 16-partition block is replicated for each of the 8 Q7 cores)."""
    idx = np.asarray(idx, dtype=np.int16)
    assert idx.size % 16 == 0
    return np.ascontiguousarray(np.tile(idx.reshape(-1, 16).T, (8, 1)))


def _round_m(m):
    for c in (4, 8, 16, 32):
        if m <= c:
            return c
    return 32


# --------------------------------------------------------------------------
# Bass program (SPMD; per-core differences enter only through input data)
# --------------------------------------------------------------------------

def build_program(emm):
    """emm: tuple of NT per-tile neighbor counts (each in {4,8,16,32})."""
    nc = bacc.Bacc(num_devices=NC)
    ikv_cols = sum(P * m for m in emm) // 16

    dp = lambda nm, shp, dt: nc.declare_dram_parameter(nm, list(shp), dt, isOutput=False)

    # ---- weights (same on all cores) ----
    type_tab = dp("type_tab", [T, H // 2], BF16)
    cat_tab = dp("cat_tab", [V * V, 2 * CD], BF16)
    dw = dp("dw", [1, H // 4], F32)
    db = dp("db", [H // 4], F32)
    projWT = dp("projWT", [3, P, H], BF16)          # proj_w.T row-chunks (padded)
    pb_col = dp("pb_col", [2, P], F32)              # proj_b as 2 columns
    wqT = dp("wqT", [L, 2, P, H], BF16)             # (wq*scale).T row-chunks
    bq = dp("bq", [L, H], F32)                      # bq*scale
    wkvT = dp("wkvT", [L, 2, P, 2 * H], BF16)       # [wk.T | wv.T] row-chunks
    woTb = dp("woTb", [L, 2, 2, P, P], BF16)        # wo.T [oc][xc] blocks
    bo_col = dp("bo_col", [L, 2, P], F32)           # (out_b + wo@bv) as columns
    woT = dp("woT", [2, P, H], BF16)                # layer-2 wo.T row-chunks
    bo_row = dp("bo_row", [H], F32)                 # layer-2 bias row
    w1T = dp("w1T", [4, P, H], BF16)
    w1eT = dp("w1eT", [2, H], BF16)
    b1 = dp("b1", [P, 2], F32)
    w2T = dp("w2T", [2, P, H // 2], BF16)
    b2 = dp("b2", [H // 2], F32)
    w3T = dp("w3T", [P, 1], BF16)
    b3 = dp("b3", [1], F32)

    # ---- per-core data ----
    idx_kv = dp("idx_kv", [P, ikv_cols], I16)       # m-major ctx idx per tile
    idx_type = dp("idx_type", [P, NL // 16], I16)
    idx_cat = dp("idx_cat", [P, NL // 16], I16)
    idx_u = dp("idx_u", [P, EL // 16], I16)
    idx_v = dp("idx_v", [P, EL // 16], I16)
    logd = dp("logd", [1, NL], F32)
    kp = dp("kp", [NL, M], F32)                     # additive pad mask (0 / NEG)
    efT = dp("efT", [2, EL], BF16)

    out_d = nc.declare_dram_parameter("out", [EL], F32, isOutput=True)

    # ---- internal DRAM ----
    kvloc = nc.dram_tensor("kvloc", [NL, 2 * H], BF16)
    xloc = nc.dram_tensor("xloc", [NL, H], BF16)
    kvall = nc.dram_tensor("kvall", [N, 2 * H], BF16, addr_space="Shared")
    xall = nc.dram_tensor("xall", [N, H], BF16, addr_space="Shared")

    groups = [list(range(NC))]
    Alu = mybir.AluOpType
    Act = mybir.ActivationFunctionType

    with tile.TileContext(nc) as tc, ExitStack() as ctx:
        const = ctx.enter_context(tc.tile_pool(name="const", bufs=1))
        xpool = ctx.enter_context(tc.tile_pool(name="xpool", bufs=1))

        gather = nc.gpsimd.dma_gather
        reg_nl = nc.gpsimd.to_reg(NL)
        reg_e2 = nc.gpsimd.to_reg(EL // 2)
        reg_pm = {m: nc.gpsimd.to_reg(P * m) for m in sorted(set(emm))}

        ident = const.tile([P, P], BF16)
        make_identity(nc, ident)

        def bcast_row(dram_ap, n, name):
            t = const.tile([P, n], F32, tag=name, name=name)
            src = bass.AP(tensor=dram_ap.tensor, offset=dram_ap.offset,
                          ap=[[0, P]] + dram_ap.ap)
            nc.sync.dma_start(out=t[:], in_=src)
            return t

        bq_b = [bcast_row(bq[ll, :], H, f"bq{ll}") for ll in range(L)]
        bo_b = bcast_row(bo_row[:], H, "bo2")

        pbc_sb = const.tile([P, 2], F32)
        nc.sync.dma_start(out=pbc_sb[:], in_=pb_col.rearrange("c p -> p c"))
        boc_sb = const.tile([P, L, 2], F32)
        nc.sync.dma_start(out=boc_sb[:], in_=bo_col.rearrange("l c p -> p l c"))

        db_sb = const.tile([H // 4, 1], F32)
        nc.sync.dma_start(out=db_sb[:], in_=db.rearrange("(p o) -> p o", o=1))
        dw_sb = const.tile([1, H // 4], F32)
        nc.sync.dma_start(out=dw_sb[:], in_=dw[:])
        b1_sb = const.tile([P, 2], F32)
        nc.sync.dma_start(out=b1_sb[:], in_=b1[:])
        b2_sb = const.tile([H // 2, 1], F32)
        nc.sync.dma_start(out=b2_sb[:], in_=b2.rearrange("(p o) -> p o", o=1))
        b3_sb = const.tile([1, 1], F32)
        nc.sync.dma_start(out=b3_sb[:], in_=b3.rearrange("(p o) -> p o", o=1))

        ikv_sb = const.tile([P, ikv_cols], I16)
        nc.sync.dma_start(out=ikv_sb[:], in_=idx_kv[:])
        ity_sb = const.tile([P, NL // 16], I16)
        nc.sync.dma_start(out=ity_sb[:], in_=idx_type[:])
        ica_sb = const.tile([P, NL // 16], I16)
        nc.sync.dma_start(out=ica_sb[:], in_=idx_cat[:])
        iu_sb = const.tile([P, EL // 16], I16)
        nc.sync.dma_start(out=iu_sb[:], in_=idx_u[:])
        iv_sb = const.tile([P, EL // 16], I16)
        nc.sync.dma_start(out=iv_sb[:], in_=idx_v[:])

        kp_sb = const.tile([P, NT, M], F32)
        nc.sync.dma_start(out=kp_sb[:], in_=kp.rearrange("(t p) m -> p t m", p=P))
        logd_sb = const.tile([1, NL], F32)
        nc.sync.dma_start(out=logd_sb[:], in_=logd[:])

        pw_sb = const.tile([P, 3, H], BF16)
        nc.sync.dma_start(out=pw_sb[:], in_=projWT.rearrange("c p o -> p c o"))
        w1_sb = const.tile([P, 4, H], BF16)
        nc.sync.dma_start(out=w1_sb[:], in_=w1T.rearrange("c p o -> p c o"))
        w1e_sb = const.tile([2, H], BF16)
        nc.sync.dma_start(out=w1e_sb[:], in_=w1eT[:])
        w2_sb = const.tile([P, 2, H // 2], BF16)
        nc.sync.dma_start(out=w2_sb[:], in_=w2T.rearrange("c p o -> p c o"))
        w3_sb = const.tile([P, 1], BF16)
        nc.sync.dma_start(out=w3_sb[:], in_=w3T[:])

        # x kept transposed: xT[:, c, g, :] = x[g-tile nodes, c*128:(c+1)*128].T
        xT = [xpool.tile([P, 2, NT, P], BF16, tag=f"xT{i}", name=f"xT{i}")
              for i in range(2)]

        # ---------------- node feature encoding ----------------
        with ExitStack() as ectx:
            enc = ectx.enter_context(tc.tile_pool(name="enc", bufs=1))
            epsum = ectx.enter_context(tc.tile_pool(name="epsum", bufs=2, space="PSUM"))
            teT = enc.tile([P, NL], BF16)
            gather(teT.rearrange("p (c n) -> p c n", c=1), type_tab[:],
                   ity_sb[:], NL, reg_nl, H // 2, transpose=True, single_packet=False)
            ccT = enc.tile([P, NL], BF16)
            gather(ccT.rearrange("p (c n) -> p c n", c=1), cat_tab[:],
                   ica_sb[:], NL, reg_nl, 2 * CD, transpose=True, single_packet=False)
            deT = enc.tile([P, NL], BF16)
            nc.vector.memset(deT[:], 0.0)
            for s in range(NL // 512):
                pd = epsum.tile([H // 4, 512], F32, tag="pdeg", name="pd")
                nc.tensor.matmul(pd[:], dw_sb[:], logd_sb[:, s * 512:(s + 1) * 512],
                                 start=True, stop=True)
                nc.scalar.activation(deT[0:H // 4, s * 512:(s + 1) * 512], pd[:],
                                     Act.Relu, bias=db_sb[:])
            featT = [teT, ccT, deT]
            for g in range(NT):
                cs = slice(g * P, (g + 1) * P)
                for xc in range(2):
                    px = epsum.tile([P, P], F32, tag="px", name="px")
                    xcs = slice(xc * P, (xc + 1) * P)
                    for c in range(3):
                        nc.tensor.matmul(px[:], pw_sb[:, c, xcs], featT[c][:, cs],
                                         start=(c == 0), stop=(c == 2))
                    nc.scalar.activation(xT[0][:, xc, g, :], px[:], Act.Identity,
                                         bias=pbc_sb[:, xc:xc + 1])

        work = ctx.enter_context(tc.tile_pool(name="work", bufs=1))
        gath = ctx.enter_context(tc.tile_pool(name="gath", bufs=2))
        att = ctx.enter_context(tc.tile_pool(name="att", bufs=2))
        psum = ctx.enter_context(tc.tile_pool(name="psum", bufs=2, space="PSUM"))
        psum1 = ctx.enter_context(tc.tile_pool(name="psum1", bufs=2, space="PSUM"))

        kvloc_pview = kvloc.rearrange("(t p) o -> p t o", p=P)
        xloc_pview = xloc.rearrange("(t p) o -> p t o", p=P)

        # ---------------- attention layers ----------------
        for ll in range(L):
            xin = xT[ll % 2]
            xout = xT[(ll + 1) % 2]
            wq_sb = work.tile([P, 2, H], BF16, tag="wq", name="wq")
            wkv_sb = work.tile([P, 2, 2 * H], BF16, tag="wkv", name="wkv")
            nc.sync.dma_start(out=wq_sb[:], in_=wqT[ll].rearrange("c p o -> p c o"))
            nc.sync.dma_start(out=wkv_sb[:], in_=wkvT[ll].rearrange("c p o -> p c o"))
            if ll < L - 1:
                wob_sb = work.tile([P, 2, 2, P], BF16, tag="wob", name="wob")
                nc.sync.dma_start(out=wob_sb[:],
                                  in_=woTb[ll].rearrange("oc xc p c -> p oc xc c"))
            else:
                wo_sb = work.tile([P, 2, H], BF16, tag="wo", name="wo")
                nc.sync.dma_start(out=wo_sb[:], in_=woT.rearrange("c p o -> p c o"))

            q_sb = work.tile([P, NT, H], BF16, tag="q", name="q_sb")
            for g in range(NT):
                pq = psum.tile([P, H], F32, tag="pq", name="pq")
                nc.tensor.matmul(pq[:], xin[:, 0, g, :], wq_sb[:, 0, :], start=True, stop=False)
                nc.tensor.matmul(pq[:], xin[:, 1, g, :], wq_sb[:, 1, :], start=False, stop=True)
                nc.vector.tensor_tensor(q_sb[:, g, :], pq[:], bq_b[ll][:], op=Alu.add)
                pkv = psum.tile([P, 2 * H], F32, tag="pkv", name="pkv")
                nc.tensor.matmul(pkv[:], xin[:, 0, g, :], wkv_sb[:, 0, :], start=True, stop=False)
                nc.tensor.matmul(pkv[:], xin[:, 1, g, :], wkv_sb[:, 1, :], start=False, stop=True)
                kvev = work.tile([P, 2 * H], BF16, tag="kvev", name="kvev", bufs=2)
                nc.scalar.activation(kvev[:], pkv[:], Act.Copy)
                nc.sync.dma_start(out=kvloc_pview[:, g, :], in_=kvev[:])

            nc.gpsimd.collective_compute("AllGather", Alu.bypass, replica_groups=groups,
                                         ins=[kvloc[:]], outs=[kvall[:]])

            ioff = 0
            for t in range(NT):
                mt = emm[t]
                isl = ikv_sb[:, ioff:ioff + P * mt // 16]
                ioff += P * mt // 16

                kvbuf = gath.tile([P, M * 2 * H], BF16, tag="kv", name="kv")
                kv = kvbuf[:, 0:mt * 2 * H].rearrange("p (m o) -> p m o", o=2 * H)
                gather(kv, kvall[:], isl, P * mt, reg_pm[mt], 2 * H,
                       single_packet=False)
                kg = kv[:, :, 0:H]
                vg = kv[:, :, H:2 * H]

                # scores: d-tree reduce, ping-pong pp<->ta
                pp = att.tile([P, M, HEADS, DH], BF16, tag="pp", name="pp", bufs=1)
                ta = att.tile([P, M, HEADS, DH // 2], BF16, tag="ta", name="ta", bufs=1)
                ppm = pp[:, 0:mt]
                tam = ta[:, 0:mt]
                qb = q_sb[:, t, None, :].to_broadcast([P, mt, H])
                nc.vector.tensor_tensor(ppm.rearrange("p m h d -> p m (h d)"),
                                        kg, qb, op=Alu.mult)
                nc.vector.tensor_tensor(tam[:], ppm[:, :, :, 0:32], ppm[:, :, :, 32:64], op=Alu.add)
                nc.vector.tensor_tensor(ppm[:, :, :, 0:16], tam[:, :, :, 0:16], tam[:, :, :, 16:32], op=Alu.add)
                nc.vector.tensor_tensor(tam[:, :, :, 0:8], ppm[:, :, :, 0:8], ppm[:, :, :, 8:16], op=Alu.add)
                nc.vector.tensor_tensor(ppm[:, :, :, 0:4], tam[:, :, :, 0:4], tam[:, :, :, 4:8], op=Alu.add)
                nc.vector.tensor_tensor(tam[:, :, :, 0:2], ppm[:, :, :, 0:2], ppm[:, :, :, 2:4], op=Alu.add)
                s_m = att.tile([P, M, HEADS], F32, tag="sm", name="s_m", bufs=1)
                smm = s_m[:, 0:mt]
                nc.vector.tensor_tensor(smm[:], tam[:, :, :, 0], tam[:, :, :, 1], op=Alu.add)
                kpb = kp_sb[:, t, 0:mt, None].to_broadcast([P, mt, HEADS])
                nc.vector.tensor_tensor(smm[:], smm[:], kpb, op=Alu.add)

                # exp + expand across DH on the Act engine (packed bf16 out)
                ee = att.tile([P, M, HEADS, DH], BF16, tag="ee", name="ee")
                eem = ee[:, 0:mt]
                nc.scalar.activation(eem[:],
                                     smm[:, :, :, None].to_broadcast([P, mt, HEADS, DH]),
                                     Act.Exp)
                sums = att.tile([P, HEADS], F32, tag="sums", name="sums", bufs=1)
                nc.vector.tensor_reduce(sums[:], eem[:, :, :, 0].rearrange("p m h -> p h m"),
                                        axis=mybir.AxisListType.X, op=Alu.add)
                rs = att.tile([P, HEADS], F32, tag="rs", name="rs", bufs=1)
                nc.vector.reciprocal(rs[:], sums[:])

                # AV: av = ee * vg (2x packed), then m-tree, then scale by rs
                av = pp.rearrange("p m h d -> p m (h d)")
                nc.vector.tensor_tensor(av[:, 0:mt], eem.rearrange("p m h d -> p m (h d)"),
                                        vg, op=Alu.mult)
                tm = ta.rearrange("p m h d -> p (m h d)").rearrange(
                    "p (m o) -> p m o", o=H)  # [P, M//2, H] scratch
                m = mt
                buf = av
                other = tm
                while m > 2:
                    m2 = m // 2
                    nc.vector.tensor_tensor(other[:, 0:m2, :], buf[:, 0:m2, :],
                                            buf[:, m2:m, :], op=Alu.add)
                    buf, other = other, buf
                    m = m2
                o_pre = att.tile([P, H], BF16, tag="o", name="o_pre", bufs=1)
                nc.vector.tensor_tensor(o_pre[:], buf[:, 0, :], buf[:, 1, :], op=Alu.add)
                o_sb = att.tile([P, H], BF16, tag="osc", name="o_sb", bufs=1)
                for h in range(HEADS):
                    hs = slice(h * DH, (h + 1) * DH)
                    nc.vector.tensor_scalar(o_sb[:, hs], o_pre[:, hs],
                                            rs[:, h:h + 1], None, op0=Alu.mult)

                # o^T via PE transpose
                oT = att.tile([P, 2, P], BF16, tag="oT", name="oT", bufs=1)
                for c in range(2):
                    pt = psum1.tile([P, P], BF16, tag="ptr", name="pt", bufs=1)
                    nc.tensor.transpose(pt[:], o_sb[:, c * P:(c + 1) * P], ident[:])
                    nc.scalar.activation(oT[:, c, :], pt[:], Act.Copy)

                if ll < L - 1:
                    # xT_next = relu(wo^T-blocks @ oT + bo) directly transposed
                    for xc in range(2):
                        pxT = psum1.tile([P, P], F32, tag="pxT", name="pxT")
                        nc.tensor.matmul(pxT[:], wob_sb[:, 0, xc, :], oT[:, 0, :],
                                         start=True, stop=False)
                        nc.tensor.matmul(pxT[:], wob_sb[:, 1, xc, :], oT[:, 1, :],
                                         start=False, stop=True)
                        nc.scalar.activation(xout[:, xc, t, :], pxT[:], Act.Relu,
                                             bias=boc_sb[:, ll, xc:xc + 1])
                else:
                    # last layer: row-layout x for the edge gather
                    pxn = psum.tile([P, H], F32, tag="pq", name="pxn")
                    nc.tensor.matmul(pxn[:], oT[:, 0, :], wo_sb[:, 0, :], start=True, stop=False)
                    nc.tensor.matmul(pxn[:], oT[:, 1, :], wo_sb[:, 1, :], start=False, stop=True)
                    xe = att.tile([P, H], BF16, tag="xe", name="xe", bufs=1)
                    nc.vector.tensor_tensor(xe[:], pxn[:], bo_b[:], op=Alu.add)
                    xev = att.tile([P, H], BF16, tag="xev", name="xev", bufs=2)
                    nc.scalar.activation(xev[:], xe[:], Act.Relu)
                    nc.sync.dma_start(out=xloc_pview[:, t, :], in_=xev[:])

        # ---------------- edge MLP ----------------
        nc.gpsimd.collective_compute("AllGather", Alu.bypass, replica_groups=groups,
                                     ins=[xloc[:]], outs=[xall[:]])

        EH = EL // 2
        for half in range(2):
            hsl = slice(half * (EH // 16), (half + 1) * (EH // 16))
            ubuf = gath.tile([P, M * 2 * H], BF16, tag="kv", name="ug")
            ug = ubuf[:, 0:2 * EH].rearrange("p (c n) -> p c n", c=2)
            gather(ug, xall[:], iu_sb[:, hsl], EH, reg_e2, H,
                   transpose=True, single_packet=False)
            vbuf = gath.tile([P, M * 2 * H], BF16, tag="kv", name="vg2")
            vg2 = vbuf[:, 0:2 * EH].rearrange("p (c n) -> p c n", c=2)
            gather(vg2, xall[:], iv_sb[:, hsl], EH, reg_e2, H,
                   transpose=True, single_packet=False)
            for e in range(EH // 512):
                eg = half * (EH // 512) + e
                esl = slice(e * 512, (e + 1) * 512)
                ef_sb = att.tile([2, 512], BF16, tag="ef", name="ef_sb")
                nc.sync.dma_start(out=ef_sb[:], in_=efT[:, eg * 512:(eg + 1) * 512])
                h1T = att.tile([P, 2, 512], BF16, tag="h1T", name="h1T")
                for oc in range(2):
                    ph = psum.tile([P, 512], F32, tag="pkv", name="ph")
                    ocs = slice(oc * P, (oc + 1) * P)
                    nc.tensor.matmul(ph[:], w1_sb[:, 0, ocs], ug[:, 0, esl], start=True, stop=False)
                    nc.tensor.matmul(ph[:], w1_sb[:, 1, ocs], ug[:, 1, esl], start=False, stop=False)
                    nc.tensor.matmul(ph[:], w1_sb[:, 2, ocs], vg2[:, 0, esl], start=False, stop=False)
                    nc.tensor.matmul(ph[:], w1_sb[:, 3, ocs], vg2[:, 1, esl], start=False, stop=False)
                    nc.tensor.matmul(ph[:], w1e_sb[:, ocs], ef_sb[:], start=False, stop=True)
                    nc.scalar.activation(h1T[:, oc, :], ph[:], Act.Relu, bias=b1_sb[:, oc:oc + 1])
                ph2 = psum.tile([P, 512], F32, tag="pkv", name="ph2")
                nc.tensor.matmul(ph2[0:H // 2, :], w2_sb[:, 0, :], h1T[:, 0, :], start=True, stop=False)
                nc.tensor.matmul(ph2[0:H // 2, :], w2_sb[:, 1, :], h1T[:, 1, :], start=False, stop=True)
                h2T = att.tile([H // 2, 512], BF16, tag="h2T", name="h2T")
                nc.scalar.activation(h2T[:], ph2[0:H // 2, :], Act.Relu, bias=b2_sb[:])
                pl = psum1.tile([1, 512], F32, tag="pl", name="pl", bufs=1)
                nc.tensor.matmul(pl[:], w3_sb[:, :], h2T[:], start=True, stop=True)
                lo = att.tile([1, 512], F32, tag="lo", name="lo")
                nc.scalar.activation(lo[:], pl[:], Act.Identity, bias=b3_sb[:])
                nc.sync.dma_start(out=out_d.rearrange("(a b) -> a b", a=ET)[eg, None, :], in_=lo[:])

    nc.finalize()
    return nc


# --------------------------------------------------------------------------
# Host-side prep + runner
# --------------------------------------------------------------------------

_CACHE = {}


def _prep_maps(inputs):
    """Returns (emm, maps): shared per-tile neighbor counts + per-core input maps."""
    f = {k: np.asarray(v) for k, v in inputs.items()}
    scale = 1.0 / np.sqrt(np.float32(DH))

    cat0, cat1 = f["cat_embed0"].astype(np.float32), f["cat_embed1"].astype(np.float32)
    cat_tab = np.zeros((V * V, 2 * CD), np.float32)
    for i0 in range(V):
        for i1 in range(V):
            cat_tab[i0 * V + i1] = np.concatenate([cat0[i0], cat1[i1]])

    projWT = np.zeros((3, P, H), np.float32)
    pwt = f["proj_w"].astype(np.float32).T  # [320, 256]
    projWT[0] = pwt[0:128]
    projWT[1] = pwt[128:256]
    projWT[2, 0:64] = pwt[256:320]
    pb_col = f["proj_b"].astype(np.float32).reshape(2, P)

    wqT = np.empty((L, 2, P, H), np.float32)
    wkvT = np.empty((L, 2, P, 2 * H), np.float32)
    woTb = np.empty((L, 2, 2, P, P), np.float32)
    bq = np.empty((L, H), np.float32)
    bo = np.empty((L, H), np.float32)
    for ll in range(L):
        w = f["in_proj_w"][ll].astype(np.float32)
        b = f["in_proj_b"][ll].astype(np.float32)
        wq, wk, wv = w[0:H], w[H:2 * H], w[2 * H:3 * H]
        bq[ll] = b[0:H] * scale
        bv = b[2 * H:3 * H]
        woTfull = f["out_w"][ll].astype(np.float32).T  # [ofeat, xfeat]
        for c in range(2):
            wqT[ll, c] = (wq * scale).T[c * P:(c + 1) * P]
            wkvT[ll, c] = np.concatenate(
                [wk.T[c * P:(c + 1) * P], wv.T[c * P:(c + 1) * P]], axis=1)
            for xc in range(2):
                woTb[ll, c, xc] = woTfull[c * P:(c + 1) * P, xc * P:(xc + 1) * P]
        bo[ll] = f["out_b"][ll].astype(np.float32) + f["out_w"][ll].astype(np.float32) @ bv
    bo_col = bo.reshape(L, 2, P)
    woT2 = np.stack([f["out_w"][L - 1].astype(np.float32).T[c * P:(c + 1) * P]
                     for c in range(2)])

    w1 = f["mlp_w1"].astype(np.float32)      # [256, 514]
    w1T_full = w1.T                           # [514, 256]
    w1T = np.stack([w1T_full[c * P:(c + 1) * P] for c in range(4)])
    w1eT = w1T_full[512:514]
    b1 = f["mlp_b1"].astype(np.float32).reshape(2, P).T  # [128, 2]
    w2T = np.stack([f["mlp_w2"].astype(np.float32).T[c * P:(c + 1) * P] for c in range(2)])
    w3T = f["mlp_w3"].astype(np.float32).T   # [128, 1]

    shared = {
        "type_tab": _bf(f["type_embed"]),
        "cat_tab": _bf(cat_tab),
        "dw": _f32(f["degree_w"].reshape(1, -1)),
        "db": _f32(f["degree_b"]),
        "projWT": _bf(projWT),
        "pb_col": _f32(pb_col),
        "wqT": _bf(wqT), "bq": _f32(bq),
        "wkvT": _bf(wkvT),
        "woTb": _bf(woTb), "bo_col": _f32(bo_col),
        "woT": _bf(woT2), "bo_row": _f32(bo[L - 1]),
        "w1T": _bf(w1T), "w1eT": _bf(w1eT), "b1": _f32(b1),
        "w2T": _bf(w2T), "b2": _f32(f["mlp_b2"]),
        "w3T": _bf(w3T), "b3": _f32(f["mlp_b3"]),
    }

    ctx = f["context_indices"].astype(np.int64)
    kpm = f["key_padding_mask"].astype(bool)
    lens = M - kpm.sum(axis=1)

    # per-core sort by context length; same-per-tile M across cores (max)
    perms, newpos = [], np.empty(N, np.int64)
    tile_max = np.zeros(NT, np.int64)
    for c in range(NC):
        ns = slice(c * NL, (c + 1) * NL)
        perm = np.argsort(lens[ns], kind="stable")  # local ranks -> local idx
        perms.append(perm)
        newpos[c * NL + perm] = c * NL + np.arange(NL)
        lsort = lens[ns][perm]
        for t in range(NT):
            tile_max[t] = max(tile_max[t], lsort[t * P:(t + 1) * P].max())
    emm = tuple(_round_m(int(m)) for m in tile_max)

    ctx_new = newpos[ctx]  # remap neighbor ids to sorted positions
    maps = []
    for c in range(NC):
        ns = slice(c * NL, (c + 1) * NL)
        es = slice(c * EL, (c + 1) * EL)
        perm = perms[c]
        ctx_c = ctx_new[ns][perm]        # [2048, 32] rows in sorted order
        idx_kv = np.concatenate(
            [ctx_c[t * P:(t + 1) * P, 0:emm[t]].T.flatten() for t in range(NT)])
        m = dict(shared)
        m["idx_kv"] = _wrap16(idx_kv)
        m["idx_type"] = _wrap16(f["type_idx"][ns][perm])
        m["idx_cat"] = _wrap16((f["cat_idx"][ns, 0] * V + f["cat_idx"][ns, 1])[perm])
        m["idx_u"] = _wrap16(newpos[f["u_idx"][es]])
        m["idx_v"] = _wrap16(newpos[f["v_idx"][es]])
        m["logd"] = _f32(f["log_degree"][ns][perm].reshape(1, NL))
        m["kp"] = _f32(np.where(kpm[ns][perm], NEG, 0.0))
        m["efT"] = _bf(f["edge_feats"][es].T)
        maps.append(m)
    return emm, maps


def kernel(**inputs):
    emm, maps = _prep_maps(inputs)
    if _CACHE.get("emm") != emm:
        _CACHE["nc"] = build_program(emm)
        _CACHE["emm"] = emm
    nc = _CACHE["nc"]
    res = run_bass_kernel_spmd(nc, maps, core_ids=list(range(NC)))
    return np.concatenate([res.results[c]["out"] for c in range(NC)]).astype(np.float32)


if __name__ == "__main__":
    emm = tuple([4, 8, 8, 16, 16, 16, 16] + [32] * 9)
    nc = build_program(emm)
    print("program built OK")


# revision 11
# speedup vs baseline: 1.8499x; 1.1816x over previous
"""Trainium2 Bass kernel for nn_EnhancedEdgeScorer (gnn_message_passing).

Sharding: data-parallel over nodes (2048/core) and edges (8192/core) on 8
NeuronCores.  Per layer, each core projects K|V (fused, one 512-wide row
per node) for its node shard, AllGathers the shard, and gathers its nodes'
neighbor K|V rows with ONE dma_gather per node tile (desc-gen on the Q7
cores is the serial bottleneck: ~8ns/index, so fusing K and V halves it).
Nodes are sorted by context length on the host and tiles use a per-tile
neighbor count M_t in {4,8,16,32}, cutting gather indices and attention
math by ~27%.

Algebraic folds:
  - k/v projected BEFORE the neighbor gather (gather commutes with the
    row-linear projection).
  - k-bias drops (softmax shift invariance); v-bias folds into the
    out-proj bias; 1/sqrt(dh) folds into wq/bq.
  - softmax normalization deferred to after the AV-sum (o *= 1/denom).
  - x kept TRANSPOSED (feature-major) between layers: projections consume
    xT directly as lhsT, the out-projection computes xT for the next layer
    directly (lhsT=wo^T blocks, rhs=o^T), and its bias+relu runs on the
    Act engine with a per-partition bias column.
  - exp runs on the Act engine with a broadcast input that also EXPANDS
    scores across the head dim, so the AV multiply reads a packed bf16
    operand (2x DVE mode) instead of a stride-0 broadcast (1x mode).
"""

import numpy as np
import ml_dtypes
from contextlib import ExitStack

import concourse.bass as bass
from concourse import bacc
import concourse.tile as tile
import concourse.mybir as mybir
from concourse.masks import make_identity
from concourse.bass_utils import run_bass_kernel_spmd

BF16 = mybir.dt.bfloat16
F32 = mybir.dt.float32
I16 = mybir.dt.int16

N, M, H, HEADS, L, E = 16384, 32, 256, 4, 3, 65536
DH = H // HEADS
T, V, CD = 8, 17, 64
TOTAL = H // 2 + 2 * CD + H // 4  # 320
NC = 8
NL = N // NC      # 2048 nodes per core
EL = E // NC      # 8192 edges per core
P = 128
NT = NL // P      # 16 node tiles per core
ET = EL // 512    # 16 edge chunks per core
NEG = -30.0       # additive pad-mask value (exp(-30) ~ 1e-13)

_bf = lambda a: np.ascontiguousarray(a.astype(ml_dtypes.bfloat16))
_f32 = lambda a: np.ascontiguousarray(a.astype(np.float32))


def _wrap16(idx):
    """Flat index list -> [128, len/16] int16 layout dma_gather expects
    (the 16-partition block is replicated for each of the 8 Q7 cores)."""
    idx = np.asarray(idx, dtype=np.int16)
    assert idx.size % 16 == 0
    return np.ascontiguousarray(np.tile(idx.reshape(-1, 16).T, (8, 1)))


def _round_m(m):
    for c in (4, 8, 16, 32):
        if m <= c:
            return c
    return 32


# --------------------------------------------------------------------------
# Bass program (SPMD; per-core differences enter only through input data)
# --------------------------------------------------------------------------

def build_program(emm):
    """emm: tuple of NT per-tile neighbor counts (each in {4,8,16,32})."""
    nc = bacc.Bacc(num_devices=NC)
    ikv_cols = sum(P * m for m in emm) // 16

    dp = lambda nm, shp, dt: nc.declare_dram_parameter(nm, list(shp), dt, isOutput=False)

    # ---- weights (same on all cores) ----
    type_tab = dp("type_tab", [T, H // 2], BF16)
    cat_tab = dp("cat_tab", [V * V, 2 * CD], BF16)
    dw = dp("dw", [1, H // 4], F32)
    db = dp("db", [H // 4], F32)
    projWT = dp("projWT", [3, P, H], BF16)          # proj_w.T row-chunks (padded)
    pb_col = dp("pb_col", [2, P], F32)              # proj_b as 2 columns
    wqT = dp("wqT", [L, 2, P, H], BF16)             # (wq*scale).T row-chunks
    bq = dp("bq", [L, H], F32)                      # bq*scale
    wkvT = dp("wkvT", [L, 2, P, 2 * H], BF16)       # [wk.T | wv.T] row-chunks
    woTb = dp("woTb", [L, 2, 2, P, P], BF16)        # wo.T [oc][xc] blocks
    bo_col = dp("bo_col", [L, 2, P], F32)           # (out_b + wo@bv) as columns
    woT = dp("woT", [2, P, H], BF16)                # layer-2 wo.T row-chunks
    bo_row = dp("bo_row", [H], F32)                 # layer-2 bias row
    w1T = dp("w1T", [4, P, H], BF16)
    w1eT = dp("w1eT", [2, H], BF16)
    b1 = dp("b1", [P, 2], F32)
    w2T = dp("w2T", [2, P, H // 2], BF16)
    b2 = dp("b2", [H // 2], F32)
    w3T = dp("w3T", [P, 1], BF16)
    b3 = dp("b3", [1], F32)

    # ---- per-core data ----
    idx_kv = dp("idx_kv", [P, ikv_cols], I16)       # m-major ctx idx per tile
    idx_type = dp("idx_type", [P, NL // 16], I16)
    idx_cat = dp("idx_cat", [P, NL // 16], I16)
    idx_u = dp("idx_u", [P, EL // 16], I16)
    idx_v = dp("idx_v", [P, EL // 16], I16)
    logd = dp("logd", [1, NL], F32)
    kp = dp("kp", [NL, M], F32)                     # additive pad mask (0 / NEG)
    efT = dp("efT", [2, EL], BF16)

    out_d = nc.declare_dram_parameter("out", [EL], F32, isOutput=True)

    # ---- internal DRAM ----
    kvloc = nc.dram_tensor("kvloc", [NL, 2 * H], BF16)
    xloc = nc.dram_tensor("xloc", [NL, H], BF16)
    kvall = nc.dram_tensor("kvall", [N, 2 * H], BF16, addr_space="Shared")
    xall = nc.dram_tensor("xall", [N, H], BF16, addr_space="Shared")

    groups = [list(range(NC))]
    Alu = mybir.AluOpType
    Act = mybir.ActivationFunctionType

    with tile.TileContext(nc) as tc, ExitStack() as ctx:
        const = ctx.enter_context(tc.tile_pool(name="const", bufs=1))
        xpool = ctx.enter_context(tc.tile_pool(name="xpool", bufs=1))

        gather = nc.gpsimd.dma_gather
        reg_nl = nc.gpsimd.to_reg(NL)
        reg_e2 = nc.gpsimd.to_reg(EL // 2)
        reg_pm = {m: nc.gpsimd.to_reg(P * m) for m in sorted(set(emm))}

        ident = const.tile([P, P], BF16)
        make_identity(nc, ident)

        def bcast_row(dram_ap, n, name):
            t = const.tile([P, n], F32, tag=name, name=name)
            src = bass.AP(tensor=dram_ap.tensor, offset=dram_ap.offset,
                          ap=[[0, P]] + dram_ap.ap)
            nc.sync.dma_start(out=t[:], in_=src)
            return t

        bq_b = [bcast_row(bq[ll, :], H, f"bq{ll}") for ll in range(L)]
        bo_b = bcast_row(bo_row[:], H, "bo2")

        pbc_sb = const.tile([P, 2], F32)
        nc.sync.dma_start(out=pbc_sb[:], in_=pb_col.rearrange("c p -> p c"))
        boc_sb = const.tile([P, L, 2], F32)
        nc.sync.dma_start(out=boc_sb[:], in_=bo_col.rearrange("l c p -> p l c"))

        db_sb = const.tile([H // 4, 1], F32)
        nc.sync.dma_start(out=db_sb[:], in_=db.rearrange("(p o) -> p o", o=1))
        dw_sb = const.tile([1, H // 4], F32)
        nc.sync.dma_start(out=dw_sb[:], in_=dw[:])
        b1_sb = const.tile([P, 2], F32)
        nc.sync.dma_start(out=b1_sb[:], in_=b1[:])
        b2_sb = const.tile([H // 2, 1], F32)
        nc.sync.dma_start(out=b2_sb[:], in_=b2.rearrange("(p o) -> p o", o=1))
        b3_sb = const.tile([1, 1], F32)
        nc.sync.dma_start(out=b3_sb[:], in_=b3.rearrange("(p o) -> p o", o=1))

        ikv_sb = const.tile([P, ikv_cols], I16)
        nc.sync.dma_start(out=ikv_sb[:], in_=idx_kv[:])
        ity_sb = const.tile([P, NL // 16], I16)
        nc.sync.dma_start(out=ity_sb[:], in_=idx_type[:])
        ica_sb = const.tile([P, NL // 16], I16)
        nc.sync.dma_start(out=ica_sb[:], in_=idx_cat[:])
        iu_sb = const.tile([P, EL // 16], I16)
        nc.sync.dma_start(out=iu_sb[:], in_=idx_u[:])
        iv_sb = const.tile([P, EL // 16], I16)
        nc.sync.dma_start(out=iv_sb[:], in_=idx_v[:])

        kp_sb = const.tile([P, NT, M], F32)
        nc.sync.dma_start(out=kp_sb[:], in_=kp.rearrange("(t p) m -> p t m", p=P))
        logd_sb = const.tile([1, NL], F32)
        nc.sync.dma_start(out=logd_sb[:], in_=logd[:])

        pw_sb = const.tile([P, 3, H], BF16)
        nc.sync.dma_start(out=pw_sb[:], in_=projWT.rearrange("c p o -> p c o"))
        w1_sb = const.tile([P, 4, H], BF16)
        nc.sync.dma_start(out=w1_sb[:], in_=w1T.rearrange("c p o -> p c o"))
        w1e_sb = const.tile([2, H], BF16)
        nc.sync.dma_start(out=w1e_sb[:], in_=w1eT[:])
        w2_sb = const.tile([P, 2, H // 2], BF16)
        nc.sync.dma_start(out=w2_sb[:], in_=w2T.rearrange("c p o -> p c o"))
        w3_sb = const.tile([P, 1], BF16)
        nc.sync.dma_start(out=w3_sb[:], in_=w3T[:])

        # x kept transposed: xT[:, c, g, :] = x[g-tile nodes, c*128:(c+1)*128].T
        xT = [xpool.tile([P, 2, NT, P], BF16, tag=f"xT{i}", name=f"xT{i}")
              for i in range(2)]

        # ---------------- node feature encoding ----------------
        with ExitStack() as ectx:
            enc = ectx.enter_context(tc.tile_pool(name="enc", bufs=1))
            epsum = ectx.enter_context(tc.tile_pool(name="epsum", bufs=2, space="PSUM"))
            teT = enc.tile([P, NL], BF16)
            gather(teT.rearrange("p (c n) -> p c n", c=1), type_tab[:],
                   ity_sb[:], NL, reg_nl, H // 2, transpose=True, single_packet=False)
            ccT = enc.tile([P, NL], BF16)
            gather(ccT.rearrange("p (c n) -> p c n", c=1), cat_tab[:],
                   ica_sb[:], NL, reg_nl, 2 * CD, transpose=True, single_packet=False)
            deT = enc.tile([P, NL], BF16)
            nc.vector.memset(deT[:], 0.0)
            for s in range(NL // 512):
                pd = epsum.tile([H // 4, 512], F32, tag="pdeg", name="pd")
                nc.tensor.matmul(pd[:], dw_sb[:], logd_sb[:, s * 512:(s + 1) * 512],
                                 start=True, stop=True)
                nc.scalar.activation(deT[0:H // 4, s * 512:(s + 1) * 512], pd[:],
                                     Act.Relu, bias=db_sb[:])
            featT = [teT, ccT, deT]
            for g in range(NT):
                cs = slice(g * P, (g + 1) * P)
                for xc in range(2):
                    px = epsum.tile([P, P], F32, tag="px", name="px")
                    xcs = slice(xc * P, (xc + 1) * P)
                    for c in range(3):
                        nc.tensor.matmul(px[:], pw_sb[:, c, xcs], featT[c][:, cs],
                                         start=(c == 0), stop=(c == 2))
                    nc.scalar.activation(xT[0][:, xc, g, :], px[:], Act.Identity,
                                         bias=pbc_sb[:, xc:xc + 1])

        work = ctx.enter_context(tc.tile_pool(name="work", bufs=1))
        gath = ctx.enter_context(tc.tile_pool(name="gath", bufs=2))
        att = ctx.enter_context(tc.tile_pool(name="att", bufs=2))
        psum = ctx.enter_context(tc.tile_pool(name="psum", bufs=2, space="PSUM"))
        psum1 = ctx.enter_context(tc.tile_pool(name="psum1", bufs=2, space="PSUM"))

        kvloc_pview = kvloc.rearrange("(t p) o -> p t o", p=P)
        xloc_pview = xloc.rearrange("(t p) o -> p t o", p=P)

        # ---------------- attention layers ----------------
        for ll in range(L):
            xin = xT[ll % 2]
            xout = xT[(ll + 1) % 2]
            wq_sb = work.tile([P, 2, H], BF16, tag="wq", name="wq")
            wkv_sb = work.tile([P, 2, 2 * H], BF16, tag="wkv", name="wkv")
            nc.sync.dma_start(out=wq_sb[:], in_=wqT[ll].rearrange("c p o -> p c o"))
            nc.sync.dma_start(out=wkv_sb[:], in_=wkvT[ll].rearrange("c p o -> p c o"))
            if ll < L - 1:
                wob_sb = work.tile([P, 2, 2, P], BF16, tag="wob", name="wob")
                nc.sync.dma_start(out=wob_sb[:],
                                  in_=woTb[ll].rearrange("oc xc p c -> p oc xc c"))
            else:
                wo_sb = work.tile([P, 2, H], BF16, tag="wo", name="wo")
                nc.sync.dma_start(out=wo_sb[:], in_=woT.rearrange("c p o -> p c o"))

            q_sb = work.tile([P, NT, H], BF16, tag="q", name="q_sb")
            for g in range(NT):
                pq = psum.tile([P, H], F32, tag="pq", name="pq")
                nc.tensor.matmul(pq[:], xin[:, 0, g, :], wq_sb[:, 0, :], start=True, stop=False)
                nc.tensor.matmul(pq[:], xin[:, 1, g, :], wq_sb[:, 1, :], start=False, stop=True)
                nc.vector.tensor_tensor(q_sb[:, g, :], pq[:], bq_b[ll][:], op=Alu.add)
                pkv = psum.tile([P, 2 * H], F32, tag="pkv", name="pkv")
                nc.tensor.matmul(pkv[:], xin[:, 0, g, :], wkv_sb[:, 0, :], start=True, stop=False)
                nc.tensor.matmul(pkv[:], xin[:, 1, g, :], wkv_sb[:, 1, :], start=False, stop=True)
                kvev = work.tile([P, 2 * H], BF16, tag="kvev", name="kvev", bufs=2)
                nc.scalar.activation(kvev[:], pkv[:], Act.Copy)
                nc.sync.dma_start(out=kvloc_pview[:, g, :], in_=kvev[:])

            nc.gpsimd.collective_compute("AllGather", Alu.bypass, replica_groups=groups,
                                         ins=[kvloc[:]], outs=[kvall[:]])

            ioff = 0
            for t in range(NT):
                mt = emm[t]
                isl = ikv_sb[:, ioff:ioff + P * mt // 16]
                ioff += P * mt // 16

                kvbuf = gath.tile([P, M * 2 * H], BF16, tag="kv", name="kv")
                kv = kvbuf[:, 0:mt * 2 * H].rearrange("p (m o) -> p m o", o=2 * H)
                gather(kv, kvall[:], isl, P * mt, reg_pm[mt], 2 * H,
                       single_packet=False)
                kg = kv[:, :, 0:H]       # [d,h]-permuted k features
                vg = kv[:, :, H:2 * H]   # standard (h,d) v features

                # scores: k is stored d-major, so every tree fold is a
                # contiguous prefix slice.  Flat buffers keep views compact
                # for tiles with mt < 32.
                pp = att.tile([P, M * H], BF16, tag="pp", name="pp", bufs=1)
                ta = att.tile([P, M * H // 2], BF16, tag="ta", name="ta", bufs=1)
                qb = q_sb[:, t, None, :].to_broadcast([P, mt, H])
                ppv = pp[:, 0:mt * H].rearrange("p (m o) -> p m o", o=H)
                nc.vector.tensor_tensor(ppv, qb, kg, op=Alu.mult)
                cur, other = pp, ta
                w = H
                while w > 8:
                    w2 = w // 2
                    cv = cur[:, 0:mt * w].rearrange("p (m o) -> p m o", o=w)
                    ov = other[:, 0:mt * w2].rearrange("p (m o) -> p m o", o=w2)
                    nc.vector.tensor_tensor(ov, cv[:, :, 0:w2], cv[:, :, w2:w], op=Alu.add)
                    cur, other = other, cur
                    w = w2
                s_m = att.tile([P, M * HEADS], F32, tag="sm", name="s_m", bufs=1)
                smm = s_m[:, 0:mt * HEADS].rearrange("p (m h) -> p m h", h=HEADS)
                cv = cur[:, 0:mt * 8].rearrange("p (m o) -> p m o", o=8)
                nc.vector.tensor_tensor(smm, cv[:, :, 0:4], cv[:, :, 4:8], op=Alu.add)
                kpb = kp_sb[:, t, 0:mt, None].to_broadcast([P, mt, HEADS])
                nc.vector.tensor_tensor(smm, smm, kpb, op=Alu.add)

                # exp + expand across DH on the Act engine (packed bf16 out)
                ee = att.tile([P, M * H], BF16, tag="ee", name="ee")
                eem = ee[:, 0:mt * H].rearrange("p (m h d) -> p m h d", h=HEADS, d=DH)
                nc.scalar.activation(eem,
                                     smm[:, :, :, None].to_broadcast([P, mt, HEADS, DH]),
                                     Act.Exp)
                sums = att.tile([P, HEADS], F32, tag="sums", name="sums", bufs=1)
                nc.vector.tensor_reduce(sums[:], eem[:, :, :, 0].rearrange("p m h -> p h m"),
                                        axis=mybir.AxisListType.X, op=Alu.add)
                rs = att.tile([P, HEADS], F32, tag="rs", name="rs", bufs=1)
                nc.vector.reciprocal(rs[:], sums[:])
                rs_exp = att.tile([P, HEADS, DH], BF16, tag="rse", name="rs_exp", bufs=1)
                nc.scalar.activation(rs_exp[:], rs[:, :, None].to_broadcast([P, HEADS, DH]),
                                     Act.Copy)

                # AV: av = ee * vg (2x packed) into the free pp buffer, then
                # contiguous m-tree halving ping-ponging pp <-> ta
                nc.vector.tensor_tensor(pp[:, 0:mt * H].rearrange("p (m o) -> p m o", o=H),
                                        eem.rearrange("p m h d -> p m (h d)"),
                                        vg, op=Alu.mult)
                m = mt
                buf, other = pp, ta
                while m > 2:
                    m2 = m // 2
                    nc.vector.tensor_tensor(other[:, 0:m2 * H], buf[:, 0:m2 * H],
                                            buf[:, m2 * H:m * H], op=Alu.add)
                    buf, other = other, buf
                    m = m2
                o_pre = att.tile([P, H], BF16, tag="o", name="o_pre", bufs=1)
                nc.vector.tensor_tensor(o_pre[:], buf[:, 0:H], buf[:, H:2 * H], op=Alu.add)
                o_sb = att.tile([P, H], BF16, tag="osc", name="o_sb", bufs=1)
                nc.vector.tensor_tensor(o_sb[:], o_pre[:],
                                        rs_exp.rearrange("p h d -> p (h d)"), op=Alu.mult)

                # o^T via PE transpose
                oT = att.tile([P, 2, P], BF16, tag="oT", name="oT", bufs=1)
                for c in range(2):
                    pt = psum1.tile([P, P], BF16, tag="ptr", name="pt", bufs=1)
                    nc.tensor.transpose(pt[:], o_sb[:, c * P:(c + 1) * P], ident[:])
                    nc.scalar.activation(oT[:, c, :], pt[:], Act.Copy)

                if ll < L - 1:
                    # xT_next = relu(wo^T-blocks @ oT + bo) directly transposed
                    for xc in range(2):
                        pxT = psum1.tile([P, P], F32, tag="pxT", name="pxT")
                        nc.tensor.matmul(pxT[:], wob_sb[:, 0, xc, :], oT[:, 0, :],
                                         start=True, stop=False)
                        nc.tensor.matmul(pxT[:], wob_sb[:, 1, xc, :], oT[:, 1, :],
                                         start=False, stop=True)
                        nc.scalar.activation(xout[:, xc, t, :], pxT[:], Act.Relu,
                                             bias=boc_sb[:, ll, xc:xc + 1])
                else:
                    # last layer: row-layout x for the edge gather
                    pxn = psum.tile([P, H], F32, tag="pq", name="pxn")
                    nc.tensor.matmul(pxn[:], oT[:, 0, :], wo_sb[:, 0, :], start=True, stop=False)
                    nc.tensor.matmul(pxn[:], oT[:, 1, :], wo_sb[:, 1, :], start=False, stop=True)
                    xe = att.tile([P, H], BF16, tag="xe", name="xe", bufs=1)
                    nc.vector.tensor_tensor(xe[:], pxn[:], bo_b[:], op=Alu.add)
                    xev = att.tile([P, H], BF16, tag="xev", name="xev", bufs=2)
                    nc.scalar.activation(xev[:], xe[:], Act.Relu)
                    nc.sync.dma_start(out=xloc_pview[:, t, :], in_=xev[:])

        # ---------------- edge MLP ----------------
        nc.gpsimd.collective_compute("AllGather", Alu.bypass, replica_groups=groups,
                                     ins=[xloc[:]], outs=[xall[:]])

        EH = EL // 2
        for half in range(2):
            hsl = slice(half * (EH // 16), (half + 1) * (EH // 16))
            ubuf = gath.tile([P, M * 2 * H], BF16, tag="kv", name="ug")
            ug = ubuf[:, 0:2 * EH].rearrange("p (c n) -> p c n", c=2)
            gather(ug, xall[:], iu_sb[:, hsl], EH, reg_e2, H,
                   transpose=True, single_packet=False)
            vbuf = gath.tile([P, M * 2 * H], BF16, tag="kv", name="vg2")
            vg2 = vbuf[:, 0:2 * EH].rearrange("p (c n) -> p c n", c=2)
            gather(vg2, xall[:], iv_sb[:, hsl], EH, reg_e2, H,
                   transpose=True, single_packet=False)
            for e in range(EH // 512):
                eg = half * (EH // 512) + e
                esl = slice(e * 512, (e + 1) * 512)
                ef_sb = att.tile([2, 512], BF16, tag="ef", name="ef_sb")
                nc.sync.dma_start(out=ef_sb[:], in_=efT[:, eg * 512:(eg + 1) * 512])
                h1T = att.tile([P, 2, 512], BF16, tag="h1T", name="h1T")
                for oc in range(2):
                    ph = psum.tile([P, 512], F32, tag="pkv", name="ph")
                    ocs = slice(oc * P, (oc + 1) * P)
                    nc.tensor.matmul(ph[:], w1_sb[:, 0, ocs], ug[:, 0, esl], start=True, stop=False)
                    nc.tensor.matmul(ph[:], w1_sb[:, 1, ocs], ug[:, 1, esl], start=False, stop=False)
                    nc.tensor.matmul(ph[:], w1_sb[:, 2, ocs], vg2[:, 0, esl], start=False, stop=False)
                    nc.tensor.matmul(ph[:], w1_sb[:, 3, ocs], vg2[:, 1, esl], start=False, stop=False)
                    nc.tensor.matmul(ph[:], w1e_sb[:, ocs], ef_sb[:], start=False, stop=True)
                    nc.scalar.activation(h1T[:, oc, :], ph[:], Act.Relu, bias=b1_sb[:, oc:oc + 1])
                ph2 = psum.tile([P, 512], F32, tag="pkv", name="ph2")
                nc.tensor.matmul(ph2[0:H // 2, :], w2_sb[:, 0, :], h1T[:, 0, :], start=True, stop=False)
                nc.tensor.matmul(ph2[0:H // 2, :], w2_sb[:, 1, :], h1T[:, 1, :], start=False, stop=True)
                h2T = att.tile([H // 2, 512], BF16, tag="h2T", name="h2T")
                nc.scalar.activation(h2T[:], ph2[0:H // 2, :], Act.Relu, bias=b2_sb[:])
                pl = psum1.tile([1, 512], F32, tag="pl", name="pl", bufs=1)
                nc.tensor.matmul(pl[:], w3_sb[:, :], h2T[:], start=True, stop=True)
                lo = att.tile([1, 512], F32, tag="lo", name="lo")
                nc.scalar.activation(lo[:], pl[:], Act.Identity, bias=b3_sb[:])
                nc.sync.dma_start(out=out_d.rearrange("(a b) -> a b", a=ET)[eg, None, :], in_=lo[:])

    nc.finalize()
    return nc


# --------------------------------------------------------------------------
# Host-side prep + runner
# --------------------------------------------------------------------------

_CACHE = {}


def _prep_maps(inputs):
    """Returns (emm, maps): shared per-tile neighbor counts + per-core input maps."""
    f = {k: np.asarray(v) for k, v in inputs.items()}
    scale = 1.0 / np.sqrt(np.float32(DH))

    cat0, cat1 = f["cat_embed0"].astype(np.float32), f["cat_embed1"].astype(np.float32)
    cat_tab = np.zeros((V * V, 2 * CD), np.float32)
    for i0 in range(V):
        for i1 in range(V):
            cat_tab[i0 * V + i1] = np.concatenate([cat0[i0], cat1[i1]])

    projWT = np.zeros((3, P, H), np.float32)
    pwt = f["proj_w"].astype(np.float32).T  # [320, 256]
    projWT[0] = pwt[0:128]
    projWT[1] = pwt[128:256]
    projWT[2, 0:64] = pwt[256:320]
    pb_col = f["proj_b"].astype(np.float32).reshape(2, P)

    # k/q features stored d-major (pos d*HEADS+h <- h*DH+d) so the score
    # tree folds are contiguous prefix slices
    perm_dh = np.array([h * DH + d for d in range(DH) for h in range(HEADS)])
    wqT = np.empty((L, 2, P, H), np.float32)
    wkvT = np.empty((L, 2, P, 2 * H), np.float32)
    woTb = np.empty((L, 2, 2, P, P), np.float32)
    bq = np.empty((L, H), np.float32)
    bo = np.empty((L, H), np.float32)
    for ll in range(L):
        w = f["in_proj_w"][ll].astype(np.float32)
        b = f["in_proj_b"][ll].astype(np.float32)
        wq, wk, wv = w[0:H], w[H:2 * H], w[2 * H:3 * H]
        bq[ll] = (b[0:H] * scale)[perm_dh]
        bv = b[2 * H:3 * H]
        woTfull = f["out_w"][ll].astype(np.float32).T  # [ofeat, xfeat]
        wqTp = (wq * scale).T[:, perm_dh]
        wkTp = wk.T[:, perm_dh]
        for c in range(2):
            wqT[ll, c] = wqTp[c * P:(c + 1) * P]
            wkvT[ll, c] = np.concatenate(
                [wkTp[c * P:(c + 1) * P], wv.T[c * P:(c + 1) * P]], axis=1)
            for xc in range(2):
                woTb[ll, c, xc] = woTfull[c * P:(c + 1) * P, xc * P:(xc + 1) * P]
        bo[ll] = f["out_b"][ll].astype(np.float32) + f["out_w"][ll].astype(np.float32) @ bv
    bo_col = bo.reshape(L, 2, P)
    woT2 = np.stack([f["out_w"][L - 1].astype(np.float32).T[c * P:(c + 1) * P]
                     for c in range(2)])

    w1 = f["mlp_w1"].astype(np.float32)      # [256, 514]
    w1T_full = w1.T                           # [514, 256]
    w1T = np.stack([w1T_full[c * P:(c + 1) * P] for c in range(4)])
    w1eT = w1T_full[512:514]
    b1 = f["mlp_b1"].astype(np.float32).reshape(2, P).T  # [128, 2]
    w2T = np.stack([f["mlp_w2"].astype(np.float32).T[c * P:(c + 1) * P] for c in range(2)])
    w3T = f["mlp_w3"].astype(np.float32).T   # [128, 1]

    shared = {
        "type_tab": _bf(f["type_embed"]),
        "cat_tab": _bf(cat_tab),
        "dw": _f32(f["degree_w"].reshape(1, -1)),
        "db": _f32(f["degree_b"]),
        "projWT": _bf(projWT),
        "pb_col": _f32(pb_col),
        "wqT": _bf(wqT), "bq": _f32(bq),
        "wkvT": _bf(wkvT),
        "woTb": _bf(woTb), "bo_col": _f32(bo_col),
        "woT": _bf(woT2), "bo_row": _f32(bo[L - 1]),
        "w1T": _bf(w1T), "w1eT": _bf(w1eT), "b1": _f32(b1),
        "w2T": _bf(w2T), "b2": _f32(f["mlp_b2"]),
        "w3T": _bf(w3T), "b3": _f32(f["mlp_b3"]),
    }

    ctx = f["context_indices"].astype(np.int64)
    kpm = f["key_padding_mask"].astype(bool)
    lens = M - kpm.sum(axis=1)

    # per-core sort by context length; same-per-tile M across cores (max)
    perms, newpos = [], np.empty(N, np.int64)
    tile_max = np.zeros(NT, np.int64)
    for c in range(NC):
        ns = slice(c * NL, (c + 1) * NL)
        perm = np.argsort(lens[ns], kind="stable")  # local ranks -> local idx
        perms.append(perm)
        newpos[c * NL + perm] = c * NL + np.arange(NL)
        lsort = lens[ns][perm]
        for t in range(NT):
            tile_max[t] = max(tile_max[t], lsort[t * P:(t + 1) * P].max())
    emm = tuple(_round_m(int(m)) for m in tile_max)

    ctx_new = newpos[ctx]  # remap neighbor ids to sorted positions
    maps = []
    for c in range(NC):
        ns = slice(c * NL, (c + 1) * NL)
        es = slice(c * EL, (c + 1) * EL)
        perm = perms[c]
        ctx_c = ctx_new[ns][perm]        # [2048, 32] rows in sorted order
        idx_kv = np.concatenate(
            [ctx_c[t * P:(t + 1) * P, 0:emm[t]].T.flatten() for t in range(NT)])
        m = dict(shared)
        m["idx_kv"] = _wrap16(idx_kv)
        m["idx_type"] = _wrap16(f["type_idx"][ns][perm])
        m["idx_cat"] = _wrap16((f["cat_idx"][ns, 0] * V + f["cat_idx"][ns, 1])[perm])
        m["idx_u"] = _wrap16(newpos[f["u_idx"][es]])
        m["idx_v"] = _wrap16(newpos[f["v_idx"][es]])
        m["logd"] = _f32(f["log_degree"][ns][perm].reshape(1, NL))
        m["kp"] = _f32(np.where(kpm[ns][perm], NEG, 0.0))
        m["efT"] = _bf(f["edge_feats"][es].T)
        maps.append(m)
    return emm, maps


def kernel(**inputs):
    emm, maps = _prep_maps(inputs)
    if _CACHE.get("emm") != emm:
        _CACHE["nc"] = build_program(emm)
        _CACHE["emm"] = emm
    nc = _CACHE["nc"]
    res = run_bass_kernel_spmd(nc, maps, core_ids=list(range(NC)))
    return np.concatenate([res.results[c]["out"] for c in range(NC)]).astype(np.float32)


if __name__ == "__main__":
    emm = tuple([4, 8, 8, 16, 16, 16, 16] + [32] * 9)
    nc = build_program(emm)
    print("program built OK")
